# revision 24
# baseline (speedup 1.0000x reference)
"""Per-sample ResNet block (conv3x3 -> relu -> conv3x3 -> +x -> relu) on 8 trn2 cores.

Full inputs: x [16,256,64,64] f32, kernel1/kernel2 [16,256,256,3,3] f32.
Sharding: pure data parallelism, 2 samples per core.

Per-core bass/tile kernel (bf16 baseline, ~287 us):
  - x sample in SBUF as two 128-channel chunks, zero-padded to 66x66 so each
    conv tap (dy,dx) is a shifted AP slice.
  - conv = sum over (ci_chunk, tap) of matmul(lhsT=kT[ci,co], rhs=x_shift)
    accumulated in PSUM (18 matmuls per [128co x 512n] tile, 216ns each at
    the N=512 streaming roofline; LDWEIGHTS fully hidden by dual buffering).
  - weights DMA'd [co, ci*9] contiguous, PE-transposed via identity.
  - relu via ScalarE activation evacuates PSUM -> SBUF; conv2 residual add
    on DVE from a resident fp32 copy of x.

"hyb" mode (default): conv1 computes FP8_TAPS of its 9 taps with fp8e4
DoubleRow matmuls (one matmul contracts all 256 input channels at the same
216ns cadence -> 2x throughput for those taps); the rest stay bf16. conv2
stays bf16. Measured on HW, a DoubleRow matmul's numerics match ml_dtypes
e4m3 simulation to ~2e-4.
  - scaling: x stored as bf16(16x), k1 as bf16(64*k1); fp8 copies are casts
    of those (so fp8 and bf16 taps share the 1024x PSUM scale), washed out
    for free by the conv1 relu eviction (relu(acc/1024)).
  - error (exact sim on the fixed seed): rel ~1.5e-2 vs 2e-2 tolerance with
    5/9 taps in fp8 (error scales ~sqrt(taps_fp8/9) * 2.15e-2).
"""

import numpy as np
from contextlib import ExitStack

import concourse.bass as bass
import concourse.mybir as mybir
import concourse.tile as tile
from concourse import bacc
from concourse.bass_utils import run_bass_kernel_spmd
from concourse.masks import make_identity

N_CORES = 8
B_FULL = 16
BPC = B_FULL // N_CORES  # samples per core
C = 256
H = W = 64
HP = WP = 66  # padded
P = 128
CCH = C // P  # channel chunks: 2
NT = 8        # spatial tiles (rows of 8) per image: 64 rows / 8
TR = 8        # rows per spatial tile
F32 = mybir.dt.float32
BF16 = mybir.dt.bfloat16
F8 = mybir.dt.float8e4
DR = mybir.MatmulPerfMode.DoubleRow
Relu = mybir.ActivationFunctionType.Relu
Copy = mybir.ActivationFunctionType.Copy

FP8_TAPS = (0, 1, 2, 3, 4, 5)   # conv1 taps computed in fp8 DoubleRow
FP8_TAPS2 = (4, 8)              # conv2 taps computed in fp8 DoubleRow
# 6 taps is the sweet spot: exact-sim rel err 1.686e-2 (vs 2e-2 gate),
# measured 250.5us. A 7th DoubleRow tap passes precision (1.786e-2) but
# tips sustained PE power over the throttle threshold: the whole matmul
# stream (incl. bf16 conv2) down-clocks 2.4->2.0GHz, 289us. A possible
# future fix: interleave conv1/conv2 tiles in a lag-4 wavefront to dilute
# MAC density to ~1.24x before adding the 7th tap.
SX = 16.0                    # x pre-scale (conv1 operands only)
SK = 64.0                    # k1 pre-scale
INV = 1.0 / (SX * SK)        # washed out at conv1 relu eviction


def build_nc(mode="hyb"):
    hyb = (mode == "hyb")
    bf_taps = tuple(t for t in range(9) if t not in FP8_TAPS) if hyb \
        else tuple(range(9))
    bf2_taps = tuple(t for t in range(9) if t not in FP8_TAPS2) if hyb \
        else tuple(range(9))
    sx = SX if hyb else 1.0
    sk = SK if hyb else 1.0
    sk2 = SK if hyb else 1.0  # conv2 weights also pre-scaled in hyb mode

    nc = bacc.Bacc("TRN2", target_bir_lowering=False, debug=False)

    x_d = nc.dram_tensor("x", [BPC, C, H, W], F32, kind="ExternalInput")
    k1_d = nc.dram_tensor("kernel1", [BPC, C, C, 3, 3], F32, kind="ExternalInput")
    k2_d = nc.dram_tensor("kernel2", [BPC, C, C, 3, 3], F32, kind="ExternalInput")
    out_d = nc.dram_tensor("out", [BPC, C, H, W], F32, kind="ExternalOutput")

    with tile.TileContext(nc) as tc, ExitStack() as ctx:
        persist = ctx.enter_context(tc.tile_pool(name="persist", bufs=1))
        kraw_p = ctx.enter_context(tc.tile_pool(name="kraw", bufs=2))
        acc_p = ctx.enter_context(tc.tile_pool(name="acc", bufs=4, space="PSUM"))
        tr_p = ctx.enter_context(tc.tile_pool(name="tr", bufs=4, space="PSUM"))
        out_p = ctx.enter_context(tc.tile_pool(name="outs", bufs=4))

        # junk operands for PE warm-up matmuls (HAM clock-gate: PE idles at
        # 1.2GHz until ~3.4us of sustained matmul activity; burn that window
        # on dataless matmuls during the initial k1/x DMA wait so the first
        # real matmuls run at 2.4GHz)
        jkw = persist.tile([P, P], BF16, tag="jkw", name="jkw")
        jk = persist.tile([P, TR, W], BF16, tag="jk", name="jk")
        nc.gpsimd.memset(jkw[:], 0.0)
        nc.gpsimd.memset(jk[:], 0.0)

        ident = persist.tile([P, P], BF16, tag="ident", name="ident")
        make_identity(nc, ident)

        for i in range(16):
            jacc = acc_p.tile([P, TR, W], F32, tag="acc", name=f"jacc{i}")
            nc.tensor.matmul(jacc[:], jkw[:], jk[:], start=True, stop=True)

        # persistent padded images + transposed weights
        xp = [persist.tile([P, CCH, HP, WP], BF16, tag=f"xp{i}", name=f"xp{i}")
              for i in range(2)]
        hp = persist.tile([P, CCH, HP, WP], BF16, tag="hp", name="hp")
        # fp32 copy of x resident for the exact residual add on DVE
        xf = [persist.tile([P, CCH, H, W], F32, tag=f"xf{i}", name=f"xf{i}")
              for i in range(2)]
        k1T = persist.tile([P, CCH, CCH, 9, P], BF16, tag="k1T", name="k1T")
        k2T = persist.tile([P, CCH, CCH, 9, P], BF16, tag="k2T", name="k2T")
        if hyb:
            xp8 = [persist.tile([P, CCH, HP, WP], F8, tag=f"xp8{i}",
                                name=f"xp8{i}") for i in range(2)]
            k18T = persist.tile([P, CCH, CCH, len(FP8_TAPS), P], F8,
                                tag="k18T", name="k18T")
            hp8 = persist.tile([P, CCH, HP, WP], F8, tag="hp8", name="hp8")
            k28T = persist.tile([P, CCH, CCH, len(FP8_TAPS2), P], F8,
                                tag="k28T", name="k28T")

        def zero_borders(t):
            # gpsimd is idle and has no framework preamble backlog
            for c in range(CCH):
                nc.gpsimd.memset(t[:, c, 0, :], 0.0)
                nc.gpsimd.memset(t[:, c, HP - 1, :], 0.0)
                nc.gpsimd.memset(t[:, c, 1:HP - 1, 0], 0.0)
                nc.gpsimd.memset(t[:, c, 1:HP - 1, WP - 1], 0.0)

        def k_dma(k_d, b, coc):
            # HWDGE f32 DMA to staging, split by ci half so the cic0
            # transposes unblock after half the DMA + half the cast.
            kr = kraw_p.tile([P, C, 9], BF16, tag="kr", name="kr")
            krs = kraw_p.tile([P, C, 9], F32, tag="krs", name="krs")
            src = k_d[b, coc * P:(coc + 1) * P].rearrange(
                "co ci kh kw -> co ci (kh kw)")
            for h in range(CCH):
                nc.sync.dma_start(out=krs[:, h * P:(h + 1) * P],
                                  in_=src[:, h * P:(h + 1) * P])
            return kr, krs

        def k_cast(kr, krs, h, scale):
            # scaled f32 -> bf16 cast of one ci half, on ACT
            nc.scalar.activation(kr[:, h * P:(h + 1) * P],
                                 krs[:, h * P:(h + 1) * P], Copy, scale=scale)

        def load_k_chunk(k_d, b, coc, scale):
            kr, krs = k_dma(k_d, b, coc)
            for h in range(CCH):
                k_cast(kr, krs, h, scale)
            return kr

        def transpose_k_chunk(kr, kT, coc, cics=None, taps=None):
            # PE-transpose each [co, ci] 128x128 tap block into kT[ci, co]
            for cic in (range(CCH) if cics is None else cics):
                for t in (range(9) if taps is None else taps):
                    ptr = tr_p.tile([P, P], BF16, tag="tr", name="ptr")
                    nc.tensor.transpose(
                        ptr[:], kr[:, cic * P:(cic + 1) * P, t], ident)
                    nc.vector.tensor_copy(kT[:, cic, coc, t, :], ptr[:])

        def k8_cast(coc):
            # fp8 copies of the DoubleRow taps (both ci chunks at once)
            for ti, t in enumerate(FP8_TAPS):
                nc.scalar.activation(
                    k18T[:, :, coc, ti, :], k1T[:, :, coc, t, :], Copy)

        def k28_cast(coc):
            for ti, t in enumerate(FP8_TAPS2):
                nc.scalar.activation(
                    k28T[:, :, coc, ti, :], k2T[:, :, coc, t, :], Copy)

        def x_piece_dma(b, c, r0, r1, eng=None):
            # HWDGE f32 DMA into the resident fp32 copy. eng=gpsimd issues
            # from the (busy-with-memsets) gpsimd queue, delaying the
            # transfer so the startup-critical k1 DMA keeps the bandwidth.
            (eng or nc.sync).dma_start(
                out=xf[b % 2][:, c, r0:r1, :],
                in_=x_d[b, c * P:(c + 1) * P, r0:r1],
            )

        def x_piece_cast(x_pad, b, c, r0, r1, on_act=False):
            # pad-insert + scaled cast to bf16; startup-critical pieces go on
            # ACT, whose queue is free while DVE drains its preamble
            dst = x_pad[:, c, 1 + r0:1 + r1, 1:1 + W]
            src = xf[b % 2][:, c, r0:r1, :]
            if on_act:
                nc.scalar.activation(dst, src, Copy, scale=sx)
            elif hyb:
                nc.vector.tensor_scalar_mul(dst, src, sx)
            else:
                nc.vector.tensor_copy(dst, src)

        def x8_piece(x_pad, x_pad8, c, r0, r1):
            # fp8 copy of the (already scaled) bf16 padded interior, on DVE
            nc.vector.tensor_copy(
                x_pad8[:, c, 1 + r0:1 + r1, 1:1 + W],
                x_pad[:, c, 1 + r0:1 + r1, 1:1 + W])

        def conv1_taps(acc, x_pad, x_pad8, coc, nt, cics=None, which="all",
                       first=False, last=False):
            """Emit conv1 matmuls for one acc tile. which: bf|f8|all."""
            r0 = nt * TR
            mms = []
            if which in ("bf", "all"):
                for t in bf_taps:
                    dy, dx = t // 3, t % 3
                    for cic in (range(CCH) if cics is None else cics):
                        mms.append((
                            k1T[:, cic, coc, t, :],
                            x_pad[:, cic, r0 + dy:r0 + dy + TR, dx:dx + W],
                            None))
            if hyb and which in ("f8", "all"):
                for ti, t in enumerate(FP8_TAPS):
                    dy, dx = t // 3, t % 3
                    mms.append((
                        k18T[:, :, coc, ti, :],
                        x_pad8[:, :, r0 + dy:r0 + dy + TR, dx:dx + W],
                        DR))
            for i, (lhsT, rhs, pm) in enumerate(mms):
                nc.tensor.matmul(
                    acc[:], lhsT, rhs,
                    start=(first and i == 0),
                    stop=(last and i == len(mms) - 1),
                    perf_mode=pm)

        def h_out(coc, nt, acc):
            # hyb: hp holds 16*relu(conv1) so conv2's bf16/fp8 taps share
            # the same 1024x PSUM scale as conv1 did (washed at y_out)
            r0 = nt * TR
            nc.scalar.activation(
                hp[:, coc, 1 + r0:1 + r0 + TR, 1:1 + W], acc[:],
                Relu, scale=(SX * INV if hyb else 1.0))
            if hyb:
                nc.vector.tensor_copy(
                    hp8[:, coc, 1 + r0:1 + r0 + TR, 1:1 + W],
                    hp[:, coc, 1 + r0:1 + r0 + TR, 1:1 + W])

        def emit_conv1(x_pad, x_pad8, nt_lo, nt_hi, cocs=None):
            for coc in (range(CCH) if cocs is None else cocs):
                for nt in range(nt_lo, nt_hi):
                    acc = acc_p.tile([P, TR, W], F32, tag="acc", name="acc")
                    conv1_taps(acc, x_pad, x_pad8, coc, nt,
                               first=True, last=True)
                    h_out(coc, nt, acc)

        def emit_conv2(x_pad, b):
            def y_out(coc, row0, acc, rows=TR, dma_eng=None):
                ot = out_p.tile([P, TR, W], F32, tag="ot", name="ot")
                # hyb: PSUM holds 1024*conv2 -> scaled Copy evacuation on
                # ACT, then exact-residual add on DVE, relu on ACT.
                if hyb:
                    nc.scalar.activation(
                        ot[:, :rows], acc[:], Copy, scale=INV)
                    nc.vector.tensor_add(
                        ot[:, :rows], ot[:, :rows],
                        xf[b % 2][:, coc, row0:row0 + rows, :])
                else:
                    nc.vector.tensor_add(
                        ot[:, :rows], acc[:],
                        xf[b % 2][:, coc, row0:row0 + rows, :])
                nc.scalar.activation(ot[:, :rows], ot[:, :rows], Relu)
                (dma_eng or nc.sync).dma_start(
                    out=out_d[b, coc * P:(coc + 1) * P,
                              row0:row0 + rows, :],
                    in_=ot[:, :rows],
                )

            def conv2_tile(coc, row0, rows, acc):
                mms = []
                for cic in range(CCH):
                    for t in bf2_taps:
                        dy, dx = t // 3, t % 3
                        mms.append((
                            k2T[:, cic, coc, t, :],
                            hp[:, cic, row0 + dy:row0 + dy + rows,
                               dx:dx + W],
                            None))
                if hyb:
                    for ti, t in enumerate(FP8_TAPS2):
                        dy, dx = t // 3, t % 3
                        mms.append((
                            k28T[:, :, coc, ti, :],
                            hp8[:, :, row0 + dy:row0 + dy + rows,
                                dx:dx + W],
                            DR))
                for i, (lhsT, rhs, pm) in enumerate(mms):
                    nc.tensor.matmul(
                        acc[:], lhsT, rhs,
                        start=(i == 0),
                        stop=(i == len(mms) - 1),
                        perf_mode=pm)

            for coc in range(CCH):
                for nt in range(NT):
                    r0 = nt * TR
                    last = (b == BPC - 1 and coc == CCH - 1 and nt == NT - 1)
                    if not last:
                        acc = acc_p.tile([P, TR, W], F32, tag="acc",
                                         name="acc")
                        conv2_tile(coc, r0, TR, acc)
                        y_out(coc, r0, acc)
                    else:
                        # final tile as two N=256 half-tiles: the first
                        # half's evac chain (+ ACT-issued DMA, no sync hop)
                        # overlaps the second half's matmuls, shrinking the
                        # post-compute tail
                        hr = TR // 2
                        for s in range(2):
                            acc = acc_p.tile([P, hr, W], F32, tag="acc",
                                             name="acc")
                            conv2_tile(coc, r0 + s * hr, hr, acc)
                            y_out(coc, r0 + s * hr, acc, rows=hr,
                                  dma_eng=nc.scalar)

        for b in range(BPC):
            x_pad = xp[b % 2]
            x_pad8 = xp8[b % 2] if hyb else None

            # Startup order: k1-coc0 DMA (ci-split halves) and x chunk0
            # land first (x chunk1 only needed for the cic1 sweep); ACT
            # alternates k-half casts with x-piece casts so the cic0
            # transposes and the cic0 bf sweep unblock as early as
            # possible while the PE is still burning warm-up matmuls.
            # DMA issue order interleaves the k1-coc0 halves with the first
            # x chunk0 piece so the nt0 bf sweep can start right after the
            # cic0 transposes instead of waiting behind the full k chunk.
            kr0 = kraw_p.tile([P, C, 9], BF16, tag="kr", name="kr0")
            krs0 = kraw_p.tile([P, C, 9], F32, tag="krs", name="krs0")
            ksrc = k1_d[b, 0:P].rearrange("co ci kh kw -> co ci (kh kw)")
            pieces = [(r, r + 12) for r in range(0, 36, 12)]
            # k1-coc0 h0 split by partition halves: two concurrent streams
            # halve the land time of the startup-critical first transpose
            # inputs
            nc.sync.dma_start(out=krs0[0:64, 0:P], in_=ksrc[0:64, 0:P])
            nc.sync.dma_start(out=krs0[64:P, 0:P], in_=ksrc[64:P, 0:P])
            nc.sync.dma_start(out=krs0[:, P:2 * P], in_=ksrc[:, P:2 * P])
            for r0, r1 in pieces:
                x_piece_dma(b, 0, r0, r1)
            for r0, r1 in pieces:
                x_piece_dma(b, 1, r0, r1)
            zero_borders(x_pad)
            if hyb:
                zero_borders(x_pad8)
            k_cast(kr0, krs0, 0, sk)
            transpose_k_chunk(kr0, k1T, 0, cics=[0])
            x_piece_cast(x_pad, b, 0, 0, 12, on_act=True)
            k_cast(kr0, krs0, 1, sk)
            transpose_k_chunk(kr0, k1T, 0, cics=[1])
            x_piece_cast(x_pad, b, 0, 12, 24, on_act=True)
            x_piece_cast(x_pad, b, 0, 24, 36, on_act=True)
            accs = [acc_p.tile([P, TR, W], F32, tag="acc", name=f"acc{i}")
                    for i in range(4)]
            for i, nt in enumerate([0, 1, 2, 3]):
                conv1_taps(accs[i], x_pad, x_pad8, 0, nt, cics=[0],
                           which="bf", first=True)
            for r0, r1 in pieces:
                x_piece_cast(x_pad, b, 1, r0, r1, on_act=True)
            if hyb:
                for r0, r1 in pieces:
                    x8_piece(x_pad, x_pad8, 0, r0, r1)
            for i, nt in enumerate([0, 1, 2, 3]):
                conv1_taps(accs[i], x_pad, x_pad8, 0, nt, cics=[1],
                           which="bf", last=not hyb)
                if not hyb:
                    h_out(0, nt, accs[i])
            if hyb:
                k8_cast(0)
                for r0, r1 in pieces:
                    x8_piece(x_pad, x_pad8, 1, r0, r1)
                for i, nt in enumerate([0, 1, 2, 3]):
                    conv1_taps(accs[i], x_pad, x_pad8, 0, nt,
                               which="f8", last=True)
                    h_out(0, nt, accs[i])
            kr1 = load_k_chunk(k1_d, b, 1, sk)
            transpose_k_chunk(kr1, k1T, 1)
            if hyb:
                k8_cast(1)
            for c in range(CCH):
                x_piece_dma(b, c, 36, H)
                x_piece_cast(x_pad, b, c, 36, H)
                if hyb:
                    x8_piece(x_pad, x_pad8, c, 36, H)
            if b == 0:
                zero_borders(hp)
                if hyb:
                    zero_borders(hp8)
            emit_conv1(x_pad, x_pad8, 0, 4, cocs=[1])
            emit_conv1(x_pad, x_pad8, 4, NT)

            for c in range(CCH):
                kr = load_k_chunk(k2_d, b, c, sk2)
                transpose_k_chunk(kr, k2T, c)
                if hyb:
                    k28_cast(c)
            emit_conv2(x_pad, b)

    nc.compile()
    return nc


_NC_CACHE = {}


def _get_nc(mode):
    if mode not in _NC_CACHE:
        _NC_CACHE[mode] = build_nc(mode)
    return _NC_CACHE[mode]


def kernel(x, kernel1, kernel2, _trace=False, _mode="hyb"):
    x = np.ascontiguousarray(np.asarray(x, dtype=np.float32))
    kernel1 = np.ascontiguousarray(np.asarray(kernel1, dtype=np.float32))
    kernel2 = np.ascontiguousarray(np.asarray(kernel2, dtype=np.float32))
    nc = _get_nc(_mode)
    in_maps = [
        {
            "x": x[i * BPC:(i + 1) * BPC],
            "kernel1": kernel1[i * BPC:(i + 1) * BPC],
            "kernel2": kernel2[i * BPC:(i + 1) * BPC],
        }
        for i in range(N_CORES)
    ]
    last_err = None
    for attempt in range(3):
        try:
            res = run_bass_kernel_spmd(
                nc, in_maps, list(range(N_CORES)), trace=_trace)
            break
        except Exception as e:  # transient NRT device errors recover on retry
            last_err = e
            if "UNRECOVERABLE" not in str(e) and "UNAVAILABLE" not in str(e):
                raise
    else:
        raise last_err
    out = np.concatenate([res.results[i]["out"] for i in range(N_CORES)], axis=0)
    if _trace:
        return out, res
    return out



# revision 25
# speedup vs baseline: 1.0198x; 1.0198x over previous
"""Per-sample ResNet block (conv3x3 -> relu -> conv3x3 -> +x -> relu) on 8 trn2 cores.

Full inputs: x [16,256,64,64] f32, kernel1/kernel2 [16,256,256,3,3] f32.
Sharding: pure data parallelism, 2 samples per core.

Per-core bass/tile kernel (bf16 baseline, ~287 us):
  - x sample in SBUF as two 128-channel chunks, zero-padded to 66x66 so each
    conv tap (dy,dx) is a shifted AP slice.
  - conv = sum over (ci_chunk, tap) of matmul(lhsT=kT[ci,co], rhs=x_shift)
    accumulated in PSUM (18 matmuls per [128co x 512n] tile, 216ns each at
    the N=512 streaming roofline; LDWEIGHTS fully hidden by dual buffering).
  - weights DMA'd [co, ci*9] contiguous, PE-transposed via identity.
  - relu via ScalarE activation evacuates PSUM -> SBUF; conv2 residual add
    on DVE from a resident fp32 copy of x.

"hyb" mode (default): conv1 computes FP8_TAPS of its 9 taps with fp8e4
DoubleRow matmuls (one matmul contracts all 256 input channels at the same
216ns cadence -> 2x throughput for those taps); the rest stay bf16. conv2
stays bf16. Measured on HW, a DoubleRow matmul's numerics match ml_dtypes
e4m3 simulation to ~2e-4.
  - scaling: x stored as bf16(16x), k1 as bf16(64*k1); fp8 copies are casts
    of those (so fp8 and bf16 taps share the 1024x PSUM scale), washed out
    for free by the conv1 relu eviction (relu(acc/1024)).
  - error (exact sim on the fixed seed): rel ~1.5e-2 vs 2e-2 tolerance with
    5/9 taps in fp8 (error scales ~sqrt(taps_fp8/9) * 2.15e-2).
"""

import numpy as np
from contextlib import ExitStack

import concourse.bass as bass
import concourse.mybir as mybir
import concourse.tile as tile
from concourse import bacc
from concourse.bass_utils import run_bass_kernel_spmd
from concourse.masks import make_identity

N_CORES = 8
B_FULL = 16
BPC = B_FULL // N_CORES  # samples per core
C = 256
H = W = 64
HP = WP = 66  # padded
P = 128
CCH = C // P  # channel chunks: 2
NT = 8        # spatial tiles (rows of 8) per image: 64 rows / 8
TR = 8        # rows per spatial tile
F32 = mybir.dt.float32
BF16 = mybir.dt.bfloat16
F8 = mybir.dt.float8e4
DR = mybir.MatmulPerfMode.DoubleRow
Relu = mybir.ActivationFunctionType.Relu
Copy = mybir.ActivationFunctionType.Copy

FP8_TAPS = (0, 1, 2, 3, 4, 5)   # conv1 taps computed in fp8 DoubleRow
FP8_TAPS2 = (4, 8)              # conv2 taps computed in fp8 DoubleRow
# 6 taps is the sweet spot: exact-sim rel err 1.686e-2 (vs 2e-2 gate),
# measured 250.5us. A 7th DoubleRow tap passes precision (1.786e-2) but
# tips sustained PE power over the throttle threshold: the whole matmul
# stream (incl. bf16 conv2) down-clocks 2.4->2.0GHz, 289us. A possible
# future fix: interleave conv1/conv2 tiles in a lag-4 wavefront to dilute
# MAC density to ~1.24x before adding the 7th tap.
SX = 16.0                    # x pre-scale (conv1 operands only)
SK = 64.0                    # k1 pre-scale
INV = 1.0 / (SX * SK)        # washed out at conv1 relu eviction


def build_nc(mode="hyb"):
    hyb = (mode == "hyb")
    bf_taps = tuple(t for t in range(9) if t not in FP8_TAPS) if hyb \
        else tuple(range(9))
    bf2_taps = tuple(t for t in range(9) if t not in FP8_TAPS2) if hyb \
        else tuple(range(9))
    sx = SX if hyb else 1.0
    sk = SK if hyb else 1.0
    sk2 = SK if hyb else 1.0  # conv2 weights also pre-scaled in hyb mode

    nc = bacc.Bacc("TRN2", target_bir_lowering=False, debug=False)

    x_d = nc.dram_tensor("x", [BPC, C, H, W], F32, kind="ExternalInput")
    k1_d = nc.dram_tensor("kernel1", [BPC, C, C, 3, 3], F32, kind="ExternalInput")
    k2_d = nc.dram_tensor("kernel2", [BPC, C, C, 3, 3], F32, kind="ExternalInput")
    out_d = nc.dram_tensor("out", [BPC, C, H, W], F32, kind="ExternalOutput")

    with tile.TileContext(nc) as tc, ExitStack() as ctx:
        persist = ctx.enter_context(tc.tile_pool(name="persist", bufs=1))
        kraw_p = ctx.enter_context(tc.tile_pool(name="kraw", bufs=2))
        acc_p = ctx.enter_context(tc.tile_pool(name="acc", bufs=4, space="PSUM"))
        tr_p = ctx.enter_context(tc.tile_pool(name="tr", bufs=4, space="PSUM"))
        out_p = ctx.enter_context(tc.tile_pool(name="outs", bufs=4))

        # junk operands for PE warm-up matmuls (HAM clock-gate: PE idles at
        # 1.2GHz until ~3.4us of sustained matmul activity; burn that window
        # on dataless matmuls during the initial k1/x DMA wait so the first
        # real matmuls run at 2.4GHz)
        jkw = persist.tile([P, P], BF16, tag="jkw", name="jkw")
        jk = persist.tile([P, TR, W], BF16, tag="jk", name="jk")
        nc.gpsimd.memset(jkw[:], 0.0)
        nc.gpsimd.memset(jk[:], 0.0)

        ident = persist.tile([P, P], BF16, tag="ident", name="ident")
        make_identity(nc, ident)

        for i in range(16):
            jacc = acc_p.tile([P, TR, W], F32, tag="acc", name=f"jacc{i}")
            nc.tensor.matmul(jacc[:], jkw[:], jk[:], start=True, stop=True)

        # persistent padded images + transposed weights
        xp = [persist.tile([P, CCH, HP, WP], BF16, tag=f"xp{i}", name=f"xp{i}")
              for i in range(2)]
        hp = persist.tile([P, CCH, HP, WP], BF16, tag="hp", name="hp")
        # fp32 copy of x resident for the exact residual add on DVE
        xf = [persist.tile([P, CCH, H, W], F32, tag=f"xf{i}", name=f"xf{i}")
              for i in range(2)]
        k1T = persist.tile([P, CCH, CCH, 9, P], BF16, tag="k1T", name="k1T")
        k2T = persist.tile([P, CCH, CCH, 9, P], BF16, tag="k2T", name="k2T")
        if hyb:
            xp8 = [persist.tile([P, CCH, HP, WP], F8, tag=f"xp8{i}",
                                name=f"xp8{i}") for i in range(2)]
            k18T = persist.tile([P, CCH, CCH, len(FP8_TAPS), P], F8,
                                tag="k18T", name="k18T")
            hp8 = persist.tile([P, CCH, HP, WP], F8, tag="hp8", name="hp8")
            k28T = persist.tile([P, CCH, CCH, len(FP8_TAPS2), P], F8,
                                tag="k28T", name="k28T")

        def zero_borders(t):
            # gpsimd is idle and has no framework preamble backlog
            for c in range(CCH):
                nc.gpsimd.memset(t[:, c, 0, :], 0.0)
                nc.gpsimd.memset(t[:, c, HP - 1, :], 0.0)
                nc.gpsimd.memset(t[:, c, 1:HP - 1, 0], 0.0)
                nc.gpsimd.memset(t[:, c, 1:HP - 1, WP - 1], 0.0)

        def k_dma(k_d, b, coc):
            # HWDGE f32 DMA to staging, split by ci half so the cic0
            # transposes unblock after half the DMA + half the cast.
            kr = kraw_p.tile([P, C, 9], BF16, tag="kr", name="kr")
            krs = kraw_p.tile([P, C, 9], F32, tag="krs", name="krs")
            src = k_d[b, coc * P:(coc + 1) * P].rearrange(
                "co ci kh kw -> co ci (kh kw)")
            for h in range(CCH):
                nc.sync.dma_start(out=krs[:, h * P:(h + 1) * P],
                                  in_=src[:, h * P:(h + 1) * P])
            return kr, krs

        def k_cast(kr, krs, h, scale):
            # scaled f32 -> bf16 cast of one ci half, on ACT
            nc.scalar.activation(kr[:, h * P:(h + 1) * P],
                                 krs[:, h * P:(h + 1) * P], Copy, scale=scale)

        def load_k_chunk(k_d, b, coc, scale):
            kr, krs = k_dma(k_d, b, coc)
            for h in range(CCH):
                k_cast(kr, krs, h, scale)
            return kr

        def transpose_k_chunk(kr, kT, coc, cics=None, taps=None):
            # PE-transpose each [co, ci] 128x128 tap block into kT[ci, co]
            for cic in (range(CCH) if cics is None else cics):
                for t in (range(9) if taps is None else taps):
                    ptr = tr_p.tile([P, P], BF16, tag="tr", name="ptr")
                    nc.tensor.transpose(
                        ptr[:], kr[:, cic * P:(cic + 1) * P, t], ident)
                    nc.vector.tensor_copy(kT[:, cic, coc, t, :], ptr[:])

        def k8_cast(coc):
            # fp8 copies of the DoubleRow taps (both ci chunks at once)
            for ti, t in enumerate(FP8_TAPS):
                nc.scalar.activation(
                    k18T[:, :, coc, ti, :], k1T[:, :, coc, t, :], Copy)

        def k28_cast(coc):
            for ti, t in enumerate(FP8_TAPS2):
                nc.scalar.activation(
                    k28T[:, :, coc, ti, :], k2T[:, :, coc, t, :], Copy)

        def x_piece_dma(b, c, r0, r1, eng=None):
            # HWDGE f32 DMA into the resident fp32 copy. eng=gpsimd issues
            # from the (busy-with-memsets) gpsimd queue, delaying the
            # transfer so the startup-critical k1 DMA keeps the bandwidth.
            (eng or nc.sync).dma_start(
                out=xf[b % 2][:, c, r0:r1, :],
                in_=x_d[b, c * P:(c + 1) * P, r0:r1],
            )

        def x_piece_cast(x_pad, b, c, r0, r1, on_act=False):
            # pad-insert + scaled cast to bf16; startup-critical pieces go on
            # ACT, whose queue is free while DVE drains its preamble
            dst = x_pad[:, c, 1 + r0:1 + r1, 1:1 + W]
            src = xf[b % 2][:, c, r0:r1, :]
            if on_act:
                nc.scalar.activation(dst, src, Copy, scale=sx)
            elif hyb:
                nc.vector.tensor_scalar_mul(dst, src, sx)
            else:
                nc.vector.tensor_copy(dst, src)

        def x8_piece(x_pad, x_pad8, c, r0, r1):
            # fp8 copy of the (already scaled) bf16 padded interior, on DVE
            nc.vector.tensor_copy(
                x_pad8[:, c, 1 + r0:1 + r1, 1:1 + W],
                x_pad[:, c, 1 + r0:1 + r1, 1:1 + W])

        def conv1_taps(acc, x_pad, x_pad8, coc, nt, cics=None, which="all",
                       first=False, last=False):
            """Emit conv1 matmuls for one acc tile. which: bf|f8|all."""
            r0 = nt * TR
            mms = []
            if which in ("bf", "all"):
                for t in bf_taps:
                    dy, dx = t // 3, t % 3
                    for cic in (range(CCH) if cics is None else cics):
                        mms.append((
                            k1T[:, cic, coc, t, :],
                            x_pad[:, cic, r0 + dy:r0 + dy + TR, dx:dx + W],
                            None))
            if hyb and which in ("f8", "all"):
                for ti, t in enumerate(FP8_TAPS):
                    dy, dx = t // 3, t % 3
                    mms.append((
                        k18T[:, :, coc, ti, :],
                        x_pad8[:, :, r0 + dy:r0 + dy + TR, dx:dx + W],
                        DR))
            for i, (lhsT, rhs, pm) in enumerate(mms):
                nc.tensor.matmul(
                    acc[:], lhsT, rhs,
                    start=(first and i == 0),
                    stop=(last and i == len(mms) - 1),
                    perf_mode=pm)

        def h_out(coc, nt, acc):
            # hyb: hp holds 16*relu(conv1) so conv2's bf16/fp8 taps share
            # the same 1024x PSUM scale as conv1 did (washed at y_out)
            r0 = nt * TR
            nc.scalar.activation(
                hp[:, coc, 1 + r0:1 + r0 + TR, 1:1 + W], acc[:],
                Relu, scale=(SX * INV if hyb else 1.0))
            if hyb:
                nc.vector.tensor_copy(
                    hp8[:, coc, 1 + r0:1 + r0 + TR, 1:1 + W],
                    hp[:, coc, 1 + r0:1 + r0 + TR, 1:1 + W])

        def emit_conv1(x_pad, x_pad8, nt_lo, nt_hi, cocs=None):
            for coc in (range(CCH) if cocs is None else cocs):
                for nt in range(nt_lo, nt_hi):
                    acc = acc_p.tile([P, TR, W], F32, tag="acc", name="acc")
                    conv1_taps(acc, x_pad, x_pad8, coc, nt,
                               first=True, last=True)
                    h_out(coc, nt, acc)

        def emit_conv2(x_pad, b):
            def y_out(coc, row0, acc, rows=TR, dma_eng=None):
                ot = out_p.tile([P, TR, W], F32, tag="ot", name="ot")
                # hyb: PSUM holds 1024*conv2 -> scaled Copy evacuation on
                # ACT, then exact-residual add on DVE, relu on ACT.
                if hyb:
                    nc.scalar.activation(
                        ot[:, :rows], acc[:], Copy, scale=INV)
                    nc.vector.tensor_add(
                        ot[:, :rows], ot[:, :rows],
                        xf[b % 2][:, coc, row0:row0 + rows, :])
                else:
                    nc.vector.tensor_add(
                        ot[:, :rows], acc[:],
                        xf[b % 2][:, coc, row0:row0 + rows, :])
                nc.scalar.activation(ot[:, :rows], ot[:, :rows], Relu)
                (dma_eng or nc.sync).dma_start(
                    out=out_d[b, coc * P:(coc + 1) * P,
                              row0:row0 + rows, :],
                    in_=ot[:, :rows],
                )

            def conv2_tile(coc, row0, rows, acc):
                mms = []
                for cic in range(CCH):
                    for t in bf2_taps:
                        dy, dx = t // 3, t % 3
                        mms.append((
                            k2T[:, cic, coc, t, :],
                            hp[:, cic, row0 + dy:row0 + dy + rows,
                               dx:dx + W],
                            None))
                if hyb:
                    for ti, t in enumerate(FP8_TAPS2):
                        dy, dx = t // 3, t % 3
                        mms.append((
                            k28T[:, :, coc, ti, :],
                            hp8[:, :, row0 + dy:row0 + dy + rows,
                                dx:dx + W],
                            DR))
                for i, (lhsT, rhs, pm) in enumerate(mms):
                    nc.tensor.matmul(
                        acc[:], lhsT, rhs,
                        start=(i == 0),
                        stop=(i == len(mms) - 1),
                        perf_mode=pm)

            for coc in range(CCH):
                for nt in range(NT):
                    r0 = nt * TR
                    last = (b == BPC - 1 and coc == CCH - 1 and nt == NT - 1)
                    if not last:
                        acc = acc_p.tile([P, TR, W], F32, tag="acc",
                                         name="acc")
                        conv2_tile(coc, r0, TR, acc)
                        y_out(coc, r0, acc)
                    else:
                        # final tile as two N=256 half-tiles: the first
                        # half's evac chain (+ ACT-issued DMA, no sync hop)
                        # overlaps the second half's matmuls, shrinking the
                        # post-compute tail
                        hr = TR // 2
                        for s in range(2):
                            acc = acc_p.tile([P, hr, W], F32, tag="acc",
                                             name="acc")
                            conv2_tile(coc, r0 + s * hr, hr, acc)
                            y_out(coc, r0 + s * hr, acc, rows=hr,
                                  dma_eng=nc.scalar)

        for b in range(BPC):
            x_pad = xp[b % 2]
            x_pad8 = xp8[b % 2] if hyb else None

            # Startup order: k1-coc0 DMA (ci-split halves) and x chunk0
            # land first (x chunk1 only needed for the cic1 sweep); ACT
            # alternates k-half casts with x-piece casts so the cic0
            # transposes and the cic0 bf sweep unblock as early as
            # possible while the PE is still burning warm-up matmuls.
            # DMA issue order interleaves the k1-coc0 halves with the first
            # x chunk0 piece so the nt0 bf sweep can start right after the
            # cic0 transposes instead of waiting behind the full k chunk.
            kr0 = kraw_p.tile([P, C, 9], BF16, tag="kr", name="kr0")
            krs0 = kraw_p.tile([P, C, 9], F32, tag="krs", name="krs0")
            ksrc = k1_d[b, 0:P].rearrange("co ci kh kw -> co ci (kh kw)")
            pieces = [(r, r + 12) for r in range(0, 36, 12)]
            nc.sync.dma_start(out=krs0[:, 0:P], in_=ksrc[:, 0:P])
            nc.sync.dma_start(out=krs0[:, P:2 * P], in_=ksrc[:, P:2 * P])
            for r0, r1 in pieces:
                x_piece_dma(b, 0, r0, r1)
            for r0, r1 in pieces:
                x_piece_dma(b, 1, r0, r1)
            zero_borders(x_pad)
            if hyb:
                zero_borders(x_pad8)
            k_cast(kr0, krs0, 0, sk)
            transpose_k_chunk(kr0, k1T, 0, cics=[0])
            x_piece_cast(x_pad, b, 0, 0, 12, on_act=True)
            k_cast(kr0, krs0, 1, sk)
            transpose_k_chunk(kr0, k1T, 0, cics=[1])
            x_piece_cast(x_pad, b, 0, 12, 24, on_act=True)
            x_piece_cast(x_pad, b, 0, 24, 36, on_act=True)
            accs = [acc_p.tile([P, TR, W], F32, tag="acc", name=f"acc{i}")
                    for i in range(4)]
            for i, nt in enumerate([0, 1, 2, 3]):
                conv1_taps(accs[i], x_pad, x_pad8, 0, nt, cics=[0],
                           which="bf", first=True)
            for r0, r1 in pieces:
                x_piece_cast(x_pad, b, 1, r0, r1, on_act=True)
            if hyb:
                for r0, r1 in pieces:
                    x8_piece(x_pad, x_pad8, 0, r0, r1)
            for i, nt in enumerate([0, 1, 2, 3]):
                conv1_taps(accs[i], x_pad, x_pad8, 0, nt, cics=[1],
                           which="bf", last=not hyb)
                if not hyb:
                    h_out(0, nt, accs[i])
            if hyb:
                k8_cast(0)
                for r0, r1 in pieces:
                    x8_piece(x_pad, x_pad8, 1, r0, r1)
                for i, nt in enumerate([0, 1, 2, 3]):
                    conv1_taps(accs[i], x_pad, x_pad8, 0, nt,
                               which="f8", last=True)
                    h_out(0, nt, accs[i])
            kr1 = load_k_chunk(k1_d, b, 1, sk)
            transpose_k_chunk(kr1, k1T, 1)
            if hyb:
                k8_cast(1)
            for c in range(CCH):
                x_piece_dma(b, c, 36, H)
                x_piece_cast(x_pad, b, c, 36, H)
                if hyb:
                    x8_piece(x_pad, x_pad8, c, 36, H)
            if b == 0:
                zero_borders(hp)
                if hyb:
                    zero_borders(hp8)
            emit_conv1(x_pad, x_pad8, 0, 4, cocs=[1])
            emit_conv1(x_pad, x_pad8, 4, NT)

            for c in range(CCH):
                kr = load_k_chunk(k2_d, b, c, sk2)
                transpose_k_chunk(kr, k2T, c)
                if hyb:
                    k28_cast(c)
            emit_conv2(x_pad, b)

    nc.compile()
    return nc


_NC_CACHE = {}


def _get_nc(mode):
    if mode not in _NC_CACHE:
        _NC_CACHE[mode] = build_nc(mode)
    return _NC_CACHE[mode]


def kernel(x, kernel1, kernel2, _trace=False, _mode="hyb"):
    x = np.ascontiguousarray(np.asarray(x, dtype=np.float32))
    kernel1 = np.ascontiguousarray(np.asarray(kernel1, dtype=np.float32))
    kernel2 = np.ascontiguousarray(np.asarray(kernel2, dtype=np.float32))
    nc = _get_nc(_mode)
    in_maps = [
        {
            "x": x[i * BPC:(i + 1) * BPC],
            "kernel1": kernel1[i * BPC:(i + 1) * BPC],
            "kernel2": kernel2[i * BPC:(i + 1) * BPC],
        }
        for i in range(N_CORES)
    ]
    last_err = None
    for attempt in range(3):
        try:
            res = run_bass_kernel_spmd(
                nc, in_maps, list(range(N_CORES)), trace=_trace)
            break
        except Exception as e:  # transient NRT device errors recover on retry
            last_err = e
            if "UNRECOVERABLE" not in str(e) and "UNAVAILABLE" not in str(e):
                raise
    else:
        raise last_err
    out = np.concatenate([res.results[i]["out"] for i in range(N_CORES)], axis=0)
    if _trace:
        return out, res
    return out



# revision 27
# speedup vs baseline: 1.0223x; 1.0024x over previous
"""Per-sample ResNet block (conv3x3 -> relu -> conv3x3 -> +x -> relu) on 8 trn2 cores.

Full inputs: x [16,256,64,64] f32, kernel1/kernel2 [16,256,256,3,3] f32.
Sharding: pure data parallelism, 2 samples per core.

Per-core bass/tile kernel (bf16 baseline, ~287 us):
  - x sample in SBUF as two 128-channel chunks, zero-padded to 66x66 so each
    conv tap (dy,dx) is a shifted AP slice.
  - conv = sum over (ci_chunk, tap) of matmul(lhsT=kT[ci,co], rhs=x_shift)
    accumulated in PSUM (18 matmuls per [128co x 512n] tile, 216ns each at
    the N=512 streaming roofline; LDWEIGHTS fully hidden by dual buffering).
  - weights DMA'd [co, ci*9] contiguous, PE-transposed via identity.
  - relu via ScalarE activation evacuates PSUM -> SBUF; conv2 residual add
    on DVE from a resident fp32 copy of x.

"hyb" mode (default): conv1 computes FP8_TAPS (6) of its 9 taps with fp8e4
DoubleRow matmuls (one matmul contracts all 256 input channels at the same
216ns cadence -> 2x throughput for those taps) and conv2 computes FP8_TAPS2
(2) the same way; the rest stay bf16. Measured on HW, a DoubleRow matmul's
numerics match ml_dtypes e4m3fn simulation to ~2e-4.
  - scaling: x as bf16(16x), k1/k2 as bf16(64k); h stored as bf16(16h) so
    both convs' fp8+bf16 taps share a 1024x PSUM scale, washed out at the
    conv1 relu eviction and at the conv2 ACT Copy evacuation (then exact
    fp32 residual add on DVE).
  - error (exact-sim on the fixed seed, validated vs HW to 2e-4): F6G2
    taps {0-5}/{4,8} -> rel 1.915e-2 vs 2e-2 gate. F7G1/F6G3 variants all
    exceed the gate; F7G0 ~ 1.90e-2 buys nothing over F6G1 in matmuls.
  - power: conv1 phase MAC density 1.5x, conv2 phase 1.125x (36 units over
    12+16 slots avg 1.286x) stays under the P0 sustained-power downclock
    threshold (~1.6x phased trips it: 2.4->2.0GHz for the whole stream).

Schedule (from perfetto-trace iterations, 253.8us -> ~233.5us):
  - 16 dataless warm-up matmuls right after the preamble: the HAM clock
    gate holds PE at 1.2GHz until ~3.4us of sustained matmul activity, and
    the k1/x DMA wait would otherwise be spent warming on real matmuls.
    They also bridge the feed-limited startup window; shortening them and
    starting real (DMA-starved, gappy) matmuls earlier re-trips the HAM
    MID window (idle >3.4us -> re-throttle) and loses more than it gains.
  - k1-coc0 DMA split in ci halves with per-half casts, so cic0 transposes
    start after half the load; x streams as 12-row pieces, chunk0 first.
  - startup DMA is bandwidth-bound (~3.5MB before the first tile group
    finishes): concurrently-issued DMAs share descriptor-queue service, so
    k lands ~12us, first real matmuls ~14-16.5us, all at full clock.
  - final conv2 tile computed as two N=256 half-tiles: first half's
    evac chain (ACT copy + DVE add + ACT relu + ACT-issued DMA, no sync
    hop) overlaps the second half's matmuls; post-compute tail ~5.7us of
    which ~2.8us is framework postamble.
  - PE busy ~215us at the N=512 streaming roofline (216ns/mm incl. LDW
    fully hidden; 128x128 transposes ~55-130ns), ~98.6% occupancy, single
    ~0.6us gap at the sample boundary.
"""

import numpy as np
from contextlib import ExitStack

import concourse.bass as bass
import concourse.mybir as mybir
import concourse.tile as tile
from concourse import bacc
from concourse.bass_utils import run_bass_kernel_spmd
from concourse.masks import make_identity

N_CORES = 8
B_FULL = 16
BPC = B_FULL // N_CORES  # samples per core
C = 256
H = W = 64
HP = WP = 66  # padded
P = 128
CCH = C // P  # channel chunks: 2
NT = 8        # spatial tiles (rows of 8) per image: 64 rows / 8
TR = 8        # rows per spatial tile
F32 = mybir.dt.float32
BF16 = mybir.dt.bfloat16
F8 = mybir.dt.float8e4
DR = mybir.MatmulPerfMode.DoubleRow
Relu = mybir.ActivationFunctionType.Relu
Copy = mybir.ActivationFunctionType.Copy

FP8_TAPS = (0, 1, 2, 3, 4, 5)   # conv1 taps computed in fp8 DoubleRow
FP8_TAPS2 = (4, 8)              # conv2 taps computed in fp8 DoubleRow
# 6+2 taps is the max-throughput point inside the 2e-2 error gate (sim
# 1.9168e-2, HW 1.915e-2). Every extra tap anywhere (F7G1 best 1.874e-2
# was a tap-choice outlier that regressed with any 2nd conv2 tap; F7G2 /
# F6G3 sims 2.02-2.17e-2) breaks the gate. A 7th conv1 DoubleRow tap also
# phases conv1 MAC density to 1.64x which trips the P0 sustained-power
# downclock (2.4->2.0GHz for the whole stream) unless conv1/conv2 tiles
# are interleaved.
SX = 16.0                    # x / h pre-scale (fp8-shared operands)
SK = 64.0                    # k1 / k2 pre-scale
INV = 1.0 / (SX * SK)        # washed out at relu / Copy evictions


def build_nc(mode="hyb"):
    hyb = (mode == "hyb")
    bf_taps = tuple(t for t in range(9) if t not in FP8_TAPS) if hyb \
        else tuple(range(9))
    bf2_taps = tuple(t for t in range(9) if t not in FP8_TAPS2) if hyb \
        else tuple(range(9))
    sx = SX if hyb else 1.0
    sk = SK if hyb else 1.0
    sk2 = SK if hyb else 1.0  # conv2 weights also pre-scaled in hyb mode

    nc = bacc.Bacc("TRN2", target_bir_lowering=False, debug=False)

    x_d = nc.dram_tensor("x", [BPC, C, H, W], F32, kind="ExternalInput")
    k1_d = nc.dram_tensor("kernel1", [BPC, C, C, 3, 3], F32, kind="ExternalInput")
    k2_d = nc.dram_tensor("kernel2", [BPC, C, C, 3, 3], F32, kind="ExternalInput")
    out_d = nc.dram_tensor("out", [BPC, C, H, W], F32, kind="ExternalOutput")

    with tile.TileContext(nc) as tc, ExitStack() as ctx:
        persist = ctx.enter_context(tc.tile_pool(name="persist", bufs=1))
        kraw_p = ctx.enter_context(tc.tile_pool(name="kraw", bufs=2))
        acc_p = ctx.enter_context(tc.tile_pool(name="acc", bufs=4, space="PSUM"))
        tr_p = ctx.enter_context(tc.tile_pool(name="tr", bufs=4, space="PSUM"))
        out_p = ctx.enter_context(tc.tile_pool(name="outs", bufs=4))

        # junk operands for PE warm-up matmuls (HAM clock-gate: PE idles at
        # 1.2GHz until ~3.4us of sustained matmul activity; burn that window
        # on dataless matmuls during the initial k1/x DMA wait so the first
        # real matmuls run at 2.4GHz)
        jkw = persist.tile([P, P], BF16, tag="jkw", name="jkw")
        jk = persist.tile([P, TR, W], BF16, tag="jk", name="jk")
        nc.gpsimd.memset(jkw[:], 0.0)
        nc.gpsimd.memset(jk[:], 0.0)

        ident = persist.tile([P, P], BF16, tag="ident", name="ident")
        make_identity(nc, ident)

        for i in range(16):
            jacc = acc_p.tile([P, TR, W], F32, tag="acc", name=f"jacc{i}")
            nc.tensor.matmul(jacc[:], jkw[:], jk[:], start=True, stop=True)

        # persistent padded images + transposed weights
        xp = [persist.tile([P, CCH, HP, WP], BF16, tag=f"xp{i}", name=f"xp{i}")
              for i in range(2)]
        hp = persist.tile([P, CCH, HP, WP], BF16, tag="hp", name="hp")
        # fp32 copy of x resident for the exact residual add on DVE
        xf = [persist.tile([P, CCH, H, W], F32, tag=f"xf{i}", name=f"xf{i}")
              for i in range(2)]
        k1T = persist.tile([P, CCH, CCH, 9, P], BF16, tag="k1T", name="k1T")
        k2T = persist.tile([P, CCH, CCH, 9, P], BF16, tag="k2T", name="k2T")
        if hyb:
            xp8 = [persist.tile([P, CCH, HP, WP], F8, tag=f"xp8{i}",
                                name=f"xp8{i}") for i in range(2)]
            k18T = persist.tile([P, CCH, CCH, len(FP8_TAPS), P], F8,
                                tag="k18T", name="k18T")
            hp8 = persist.tile([P, CCH, HP, WP], F8, tag="hp8", name="hp8")
            k28T = persist.tile([P, CCH, CCH, len(FP8_TAPS2), P], F8,
                                tag="k28T", name="k28T")

        def zero_borders(t):
            # gpsimd is idle and has no framework preamble backlog
            for c in range(CCH):
                nc.gpsimd.memset(t[:, c, 0, :], 0.0)
                nc.gpsimd.memset(t[:, c, HP - 1, :], 0.0)
                nc.gpsimd.memset(t[:, c, 1:HP - 1, 0], 0.0)
                nc.gpsimd.memset(t[:, c, 1:HP - 1, WP - 1], 0.0)

        def k_dma(k_d, b, coc):
            # HWDGE f32 DMA to staging, split by ci half so the cic0
            # transposes unblock after half the DMA + half the cast.
            kr = kraw_p.tile([P, C, 9], BF16, tag="kr", name="kr")
            krs = kraw_p.tile([P, C, 9], F32, tag="krs", name="krs")
            src = k_d[b, coc * P:(coc + 1) * P].rearrange(
                "co ci kh kw -> co ci (kh kw)")
            for h in range(CCH):
                nc.sync.dma_start(out=krs[:, h * P:(h + 1) * P],
                                  in_=src[:, h * P:(h + 1) * P])
            return kr, krs

        def k_cast(kr, krs, h, scale):
            # scaled f32 -> bf16 cast of one ci half, on ACT
            nc.scalar.activation(kr[:, h * P:(h + 1) * P],
                                 krs[:, h * P:(h + 1) * P], Copy, scale=scale)

        def load_k_chunk(k_d, b, coc, scale):
            kr, krs = k_dma(k_d, b, coc)
            for h in range(CCH):
                k_cast(kr, krs, h, scale)
            return kr

        def transpose_k_chunk(kr, kT, coc, cics=None, taps=None):
            # PE-transpose each [co, ci] 128x128 tap block into kT[ci, co]
            for cic in (range(CCH) if cics is None else cics):
                for t in (range(9) if taps is None else taps):
                    ptr = tr_p.tile([P, P], BF16, tag="tr", name="ptr")
                    nc.tensor.transpose(
                        ptr[:], kr[:, cic * P:(cic + 1) * P, t], ident)
                    nc.vector.tensor_copy(kT[:, cic, coc, t, :], ptr[:])

        def k8_cast(coc):
            # fp8 copies of the DoubleRow taps (both ci chunks at once)
            for ti, t in enumerate(FP8_TAPS):
                nc.scalar.activation(
                    k18T[:, :, coc, ti, :], k1T[:, :, coc, t, :], Copy)

        def k28_cast(coc):
            for ti, t in enumerate(FP8_TAPS2):
                nc.scalar.activation(
                    k28T[:, :, coc, ti, :], k2T[:, :, coc, t, :], Copy)

        def x_piece_dma(b, c, r0, r1, eng=None):
            # HWDGE f32 DMA into the resident fp32 copy. eng=gpsimd issues
            # from the (busy-with-memsets) gpsimd queue, delaying the
            # transfer so the startup-critical k1 DMA keeps the bandwidth.
            (eng or nc.sync).dma_start(
                out=xf[b % 2][:, c, r0:r1, :],
                in_=x_d[b, c * P:(c + 1) * P, r0:r1],
            )

        def x_piece_cast(x_pad, b, c, r0, r1, on_act=False):
            # pad-insert + scaled cast to bf16; startup-critical pieces go on
            # ACT, whose queue is free while DVE drains its preamble
            dst = x_pad[:, c, 1 + r0:1 + r1, 1:1 + W]
            src = xf[b % 2][:, c, r0:r1, :]
            if on_act:
                nc.scalar.activation(dst, src, Copy, scale=sx)
            elif hyb:
                nc.vector.tensor_scalar_mul(dst, src, sx)
            else:
                nc.vector.tensor_copy(dst, src)

        def x8_piece(x_pad, x_pad8, c, r0, r1):
            # fp8 copy of the (already scaled) bf16 padded interior, on DVE
            nc.vector.tensor_copy(
                x_pad8[:, c, 1 + r0:1 + r1, 1:1 + W],
                x_pad[:, c, 1 + r0:1 + r1, 1:1 + W])

        def conv1_taps(acc, x_pad, x_pad8, coc, nt, cics=None, which="all",
                       first=False, last=False):
            """Emit conv1 matmuls for one acc tile. which: bf|f8|all."""
            r0 = nt * TR
            mms = []
            if which in ("bf", "all"):
                for t in bf_taps:
                    dy, dx = t // 3, t % 3
                    for cic in (range(CCH) if cics is None else cics):
                        mms.append((
                            k1T[:, cic, coc, t, :],
                            x_pad[:, cic, r0 + dy:r0 + dy + TR, dx:dx + W],
                            None))
            if hyb and which in ("f8", "all"):
                for ti, t in enumerate(FP8_TAPS):
                    dy, dx = t // 3, t % 3
                    mms.append((
                        k18T[:, :, coc, ti, :],
                        x_pad8[:, :, r0 + dy:r0 + dy + TR, dx:dx + W],
                        DR))
            for i, (lhsT, rhs, pm) in enumerate(mms):
                nc.tensor.matmul(
                    acc[:], lhsT, rhs,
                    start=(first and i == 0),
                    stop=(last and i == len(mms) - 1),
                    perf_mode=pm)

        def h_out(coc, nt, acc):
            # hyb: hp holds 16*relu(conv1) so conv2's bf16/fp8 taps share
            # the same 1024x PSUM scale as conv1 did (washed at y_out)
            r0 = nt * TR
            nc.scalar.activation(
                hp[:, coc, 1 + r0:1 + r0 + TR, 1:1 + W], acc[:],
                Relu, scale=(SX * INV if hyb else 1.0))
            if hyb:
                nc.vector.tensor_copy(
                    hp8[:, coc, 1 + r0:1 + r0 + TR, 1:1 + W],
                    hp[:, coc, 1 + r0:1 + r0 + TR, 1:1 + W])

        def emit_conv1(x_pad, x_pad8, nt_lo, nt_hi, cocs=None):
            for coc in (range(CCH) if cocs is None else cocs):
                for nt in range(nt_lo, nt_hi):
                    acc = acc_p.tile([P, TR, W], F32, tag="acc", name="acc")
                    conv1_taps(acc, x_pad, x_pad8, coc, nt,
                               first=True, last=True)
                    h_out(coc, nt, acc)

        def emit_conv2(x_pad, b):
            def y_out(coc, row0, acc, rows=TR, dma_eng=None):
                ot = out_p.tile([P, TR, W], F32, tag="ot", name="ot")
                # hyb: PSUM holds 1024*conv2 -> scaled Copy evacuation on
                # ACT, then exact-residual add on DVE, relu on ACT.
                if hyb:
                    nc.scalar.activation(
                        ot[:, :rows], acc[:], Copy, scale=INV)
                    nc.vector.tensor_add(
                        ot[:, :rows], ot[:, :rows],
                        xf[b % 2][:, coc, row0:row0 + rows, :])
                else:
                    nc.vector.tensor_add(
                        ot[:, :rows], acc[:],
                        xf[b % 2][:, coc, row0:row0 + rows, :])
                nc.scalar.activation(ot[:, :rows], ot[:, :rows], Relu)
                (dma_eng or nc.sync).dma_start(
                    out=out_d[b, coc * P:(coc + 1) * P,
                              row0:row0 + rows, :],
                    in_=ot[:, :rows],
                )

            def conv2_tile(coc, row0, rows, acc):
                mms = []
                for cic in range(CCH):
                    for t in bf2_taps:
                        dy, dx = t // 3, t % 3
                        mms.append((
                            k2T[:, cic, coc, t, :],
                            hp[:, cic, row0 + dy:row0 + dy + rows,
                               dx:dx + W],
                            None))
                if hyb:
                    for ti, t in enumerate(FP8_TAPS2):
                        dy, dx = t // 3, t % 3
                        mms.append((
                            k28T[:, :, coc, ti, :],
                            hp8[:, :, row0 + dy:row0 + dy + rows,
                                dx:dx + W],
                            DR))
                for i, (lhsT, rhs, pm) in enumerate(mms):
                    nc.tensor.matmul(
                        acc[:], lhsT, rhs,
                        start=(i == 0),
                        stop=(i == len(mms) - 1),
                        perf_mode=pm)

            for coc in range(CCH):
                for nt in range(NT):
                    r0 = nt * TR
                    last = (b == BPC - 1 and coc == CCH - 1 and nt == NT - 1)
                    if not last:
                        acc = acc_p.tile([P, TR, W], F32, tag="acc",
                                         name="acc")
                        conv2_tile(coc, r0, TR, acc)
                        y_out(coc, r0, acc)
                    else:
                        # final tile as two N=256 half-tiles: the first
                        # half's evac chain (+ ACT-issued DMA, no sync hop)
                        # overlaps the second half's matmuls, shrinking the
                        # post-compute tail
                        hr = TR // 2
                        for s in range(2):
                            acc = acc_p.tile([P, hr, W], F32, tag="acc",
                                             name="acc")
                            conv2_tile(coc, r0 + s * hr, hr, acc)
                            y_out(coc, r0 + s * hr, acc, rows=hr,
                                  dma_eng=nc.scalar)

        for b in range(BPC):
            x_pad = xp[b % 2]
            x_pad8 = xp8[b % 2] if hyb else None

            # Startup order: k1-coc0 DMA (ci-split halves) and x chunk0
            # land first (x chunk1 only needed for the cic1 sweep); ACT
            # alternates k-half casts with x-piece casts so the cic0
            # transposes and the cic0 bf sweep unblock as early as
            # possible while the PE is still burning warm-up matmuls.
            # DMA issue order interleaves the k1-coc0 halves with the first
            # x chunk0 piece so the nt0 bf sweep can start right after the
            # cic0 transposes instead of waiting behind the full k chunk.
            kr0 = kraw_p.tile([P, C, 9], BF16, tag="kr", name="kr0")
            krs0 = kraw_p.tile([P, C, 9], F32, tag="krs", name="krs0")
            ksrc = k1_d[b, 0:P].rearrange("co ci kh kw -> co ci (kh kw)")
            pieces = [(r, r + 12) for r in range(0, 36, 12)]
            nc.sync.dma_start(out=krs0[:, 0:P], in_=ksrc[:, 0:P])
            nc.sync.dma_start(out=krs0[:, P:2 * P], in_=ksrc[:, P:2 * P])
            for r0, r1 in pieces:
                x_piece_dma(b, 0, r0, r1)
            for r0, r1 in pieces:
                x_piece_dma(b, 1, r0, r1)
            zero_borders(x_pad)
            if hyb:
                zero_borders(x_pad8)
            k_cast(kr0, krs0, 0, sk)
            transpose_k_chunk(kr0, k1T, 0, cics=[0])
            x_piece_cast(x_pad, b, 0, 0, 12, on_act=True)
            k_cast(kr0, krs0, 1, sk)
            transpose_k_chunk(kr0, k1T, 0, cics=[1])
            x_piece_cast(x_pad, b, 0, 12, 24, on_act=True)
            x_piece_cast(x_pad, b, 0, 24, 36, on_act=True)
            accs = [acc_p.tile([P, TR, W], F32, tag="acc", name=f"acc{i}")
                    for i in range(4)]
            for i, nt in enumerate([0, 1, 2, 3]):
                conv1_taps(accs[i], x_pad, x_pad8, 0, nt, cics=[0],
                           which="bf", first=True)
            for r0, r1 in pieces:
                x_piece_cast(x_pad, b, 1, r0, r1, on_act=True)
            if hyb:
                for r0, r1 in pieces:
                    x8_piece(x_pad, x_pad8, 0, r0, r1)
            for i, nt in enumerate([0, 1, 2, 3]):
                conv1_taps(accs[i], x_pad, x_pad8, 0, nt, cics=[1],
                           which="bf", last=not hyb)
                if not hyb:
                    h_out(0, nt, accs[i])
            if hyb:
                k8_cast(0)
                for r0, r1 in pieces:
                    x8_piece(x_pad, x_pad8, 1, r0, r1)
                for i, nt in enumerate([0, 1, 2, 3]):
                    conv1_taps(accs[i], x_pad, x_pad8, 0, nt,
                               which="f8", last=True)
                    h_out(0, nt, accs[i])
            kr1 = load_k_chunk(k1_d, b, 1, sk)
            transpose_k_chunk(kr1, k1T, 1)
            if hyb:
                k8_cast(1)
            for c in range(CCH):
                x_piece_dma(b, c, 36, H)
                x_piece_cast(x_pad, b, c, 36, H)
                if hyb:
                    x8_piece(x_pad, x_pad8, c, 36, H)
            if b == 0:
                zero_borders(hp)
                if hyb:
                    zero_borders(hp8)
            emit_conv1(x_pad, x_pad8, 0, 4, cocs=[1])
            emit_conv1(x_pad, x_pad8, 4, NT)

            for c in range(CCH):
                kr = load_k_chunk(k2_d, b, c, sk2)
                transpose_k_chunk(kr, k2T, c)
                if hyb:
                    k28_cast(c)
            emit_conv2(x_pad, b)

    nc.compile()
    return nc


_NC_CACHE = {}


def _get_nc(mode):
    if mode not in _NC_CACHE:
        _NC_CACHE[mode] = build_nc(mode)
    return _NC_CACHE[mode]


def kernel(x, kernel1, kernel2, _trace=False, _mode="hyb"):
    x = np.ascontiguousarray(np.asarray(x, dtype=np.float32))
    kernel1 = np.ascontiguousarray(np.asarray(kernel1, dtype=np.float32))
    kernel2 = np.ascontiguousarray(np.asarray(kernel2, dtype=np.float32))
    nc = _get_nc(_mode)
    in_maps = [
        {
            "x": x[i * BPC:(i + 1) * BPC],
            "kernel1": kernel1[i * BPC:(i + 1) * BPC],
            "kernel2": kernel2[i * BPC:(i + 1) * BPC],
        }
        for i in range(N_CORES)
    ]
    last_err = None
    for attempt in range(3):
        try:
            res = run_bass_kernel_spmd(
                nc, in_maps, list(range(N_CORES)), trace=_trace)
            break
        except Exception as e:  # transient NRT device errors recover on retry
            last_err = e
            if "UNRECOVERABLE" not in str(e) and "UNAVAILABLE" not in str(e):
                raise
    else:
        raise last_err
    out = np.concatenate([res.results[i]["out"] for i in range(N_CORES)], axis=0)
    if _trace:
        return out, res
    return out

